# revision 55
# baseline (speedup 1.0000x reference)
"""Trainium2 Bass kernel for nn_CrossAttention (dense_transformer).

Reference computation (per batch b, per stream s in {1,2}):
    q_s   = heads(x_s)                      # [H, N, D] slices of x_s
    kv_s  = x_s @ Wkv_s -> k_s, v_s         # [N, C] each
    gate_s= sigmoid(relu(x_s @ w1 + b1) @ w2 + b2)
    ctx_s = softmax_d( scale * k_s^T @ (v_s * gate_s) )   # [H, D, D]
    o_1   = q_1 @ ctx_2 ; o_2 = q_2 @ ctx_1  (cross)

Sharding: 8 cores = (stream s, batch b) pairs.  Core (s, b) projects
x_s[b] (kv + gate + ctx_s[b]) and then computes the OTHER stream's
output o_{1-s}[b] = q_{1-s}[b] @ softmax(ctx_s[b]).  No cross-core
communication; host concatenates outputs.

v2: host pre-transposes/pre-casts x (fp16), so the device does no
transposes and no DRAM spills.  All GEMMs fp16 (1 cycle/row); the gate
MLP can optionally run fp8e4 DoubleRow (0.5 cycles/row).  ctx is
accumulated in PSUM across all 32 n-chunks (two 8-head groups stacked
on partition halves -> one PSUM bank).
"""

import numpy as np
from contextlib import ExitStack

N = 4096
C = 1024
H = 16
D = 64
SCALE = D ** (-0.5)
NCH = N // 128       # 32 n-chunks of 128 rows

GATE_MODE = "fp8"    # 'fp16' | 'fp8'
S_X = 16.0           # fp8 activation scale for x
S_W = 256.0          # fp8 weight scale
S_H = 32.0           # fp8 scale for hidden h

_CACHE = {}


def _build_program(gate_mode, with_bias):
    import concourse.bass as bass
    import concourse.bacc as bacc
    import concourse.tile as tile
    import concourse.mybir as mybir

    F32 = mybir.dt.float32
    FP16 = mybir.dt.float16
    FP8 = mybir.dt.float8e4
    AF = mybir.ActivationFunctionType
    DR = mybir.MatmulPerfMode.DoubleRow
    fp8 = gate_mode == "fp8"
    HDT = FP8 if fp8 else FP16

    nc = bacc.Bacc("TRN2", target_bir_lowering=False, debug=False, num_devices=8)

    # weights arrive host-rearranged to the SBUF layout [p, k, m] so the
    # DMA is a straight contiguous copy (8-16KB lines per partition)
    xt = nc.dram_tensor("xt", [C, N], FP16, kind="ExternalInput").ap()
    xqt = nc.dram_tensor("xqt", [C, N], FP16, kind="ExternalInput").ap()
    wkv = nc.dram_tensor("wkv", [128, 8 * 2 * C], FP16, kind="ExternalInput").ap()
    w1 = nc.dram_tensor("w1", [128, 8 * C], HDT, kind="ExternalInput").ap()
    w2 = nc.dram_tensor("w2", [128, 8 * C], HDT, kind="ExternalInput").ap()
    b1s = nc.dram_tensor("b1s", [128, 8], F32, kind="ExternalInput").ap()
    ident = nc.dram_tensor("ident", [128, 64], F32, kind="ExternalInput").ap()
    if fp8:
        xt8 = nc.dram_tensor("xt8", [C, N], FP8, kind="ExternalInput").ap()
    if with_bias:
        b2r = nc.dram_tensor("b2r", [1, C], FP16, kind="ExternalInput").ap()
    o = nc.dram_tensor("o", [N, C], FP16, kind="ExternalOutput").ap()

    # activation post-scales to undo the fp8 pre-scales
    g1_scale = (S_H / (S_X * S_W)) if fp8 else 1.0
    g2_scale = (1.0 / (S_H * S_W)) if fp8 else 1.0
    ones_val = (S_H * S_W) if fp8 else 1.0

    with tile.TileContext(nc) as tc, ExitStack() as ctx:
        # ---------- persistent constants ----------
        # DMA order matters: only w1/b1 (+ first x block) gate the first
        # matmul, so emit those first and defer w2/wkv into block 0's
        # compute window.
        # big DMAs are split into k-range parts so they spread across DMA
        # queues (a single dma_start lands on one queue)
        def dma_split(dst, src, parts):
            kk = 8 // parts
            for i in range(parts):
                nc.sync.dma_start(
                    dst[:, i * kk:(i + 1) * kk, :], src[:, i * kk:(i + 1) * kk, :]
                )

        cpool = ctx.enter_context(tc.tile_pool(name="consts", bufs=1))
        w1_sb = cpool.tile([128, 8, C], HDT, name="w1_sb")
        b1_sb = cpool.tile([128, 8], F32, name="b1_sb")
        w2_sb = cpool.tile([128, 8, C], HDT, name="w2_sb")
        wkv_sb = cpool.tile([128, 8, 2 * C], FP16, name="wkv_sb")
        ident_sb = cpool.tile([128, 64], F32, name="ident_sb")

        def emit_deferred_consts():
            dma_split(w2_sb, w2.rearrange("p (k m) -> p k m", k=8), 4)
            dma_split(wkv_sb, wkv.rearrange("p (k m) -> p k m", k=8), 8)
            nc.sync.dma_start(ident_sb, ident)
        if with_bias:
            ones_sb = cpool.tile([1, 128], F32, name="ones_sb")
            nc.vector.memset(ones_sb, ones_val)
            ones_r = cpool.tile([1, 128], FP16, name="ones_r")
            nc.vector.tensor_copy(ones_r, ones_sb)
            b2_r = cpool.tile([1, C], FP16, name="b2_r")
            nc.sync.dma_start(b2_r, b2r)

        # pre-warm the scalar Exp activation table while the engine is idle
        # so the softmax doesn't pay the ~1.3us ACT_TABLE_LOAD at its start
        warm = cpool.tile([1, 16], F32, name="warm")
        nc.vector.memset(warm, 0.0)
        warm2 = cpool.tile([1, 16], F32, name="warm2")
        nc.scalar.activation(warm2, warm, AF.Exp)

        # spair off-diagonal blocks are zero forever: zero them once here,
        # off the critical path, so softmax only writes the diagonals
        spool = ctx.enter_context(tc.tile_pool(name="spairs", bufs=1))
        spairs = [spool.tile([128, 128], FP16, name=f"spair{j}") for j in range(8)]
        for j in range(8):
            nc.vector.memset(spairs[j], 0.0)

        # ctx accumulator in PSUM: heads 0-7 on partitions 0-63, heads
        # 8-15 on 64-127; head h at cols (h%8)*64, layout [e, d].
        ctxps_pool = ctx.enter_context(
            tc.tile_pool(name="ctxps", bufs=1, space="PSUM")
        )
        ctx_ps = ctxps_pool.tile([128, 512], F32, name="ctx_ps")

        # phase-B xq tiles live alongside phase A so DMA prefetch overlaps
        bxq_pool = ctx.enter_context(tc.tile_pool(name="bxq", bufs=4))

        def emit_bxq_dma(blk):
            bx = bxq_pool.tile([128, 8, 1024], FP16, name="bx", tag="bx")
            src = xqt.rearrange("(j p) n -> p j n", p=128)[
                :, :, blk * 1024:(blk + 1) * 1024
            ]
            for i in range(4):
                nc.sync.dma_start(bx[:, 2 * i:2 * i + 2, :], src[:, 2 * i:2 * i + 2, :])
            return bx

        # =========================================================
        # Phase A: gates + kv projection + ctx accumulation, fused
        # =========================================================
        with ExitStack() as pa:
            xt_pool = pa.enter_context(tc.tile_pool(name="xt", bufs=2))
            if fp8:
                xt8_pool = pa.enter_context(tc.tile_pool(name="xt8", bufs=2))
            ht_pool = pa.enter_context(tc.tile_pool(name="ht", bufs=2))
            g_pool = pa.enter_context(tc.tile_pool(name="g", bufs=3))
            kf_pool = pa.enter_context(tc.tile_pool(name="kf", bufs=3))
            vg_pool = pa.enter_context(tc.tile_pool(name="vg", bufs=3))
            gps_pool = pa.enter_context(
                tc.tile_pool(name="gps", bufs=3, space="PSUM")
            )
            kvps_pool = pa.enter_context(
                tc.tile_pool(name="kvps", bufs=2, space="PSUM")
            )

            bx_tiles = {}
            pending = []  # (kf, vg, global_chunk) awaiting ctx matmuls

            def emit_ctx(kf_t, vg_t, gc):
                # start=True marks the whole 2KB PSUM bank (per partition)
                # as pending-zero, so issue it exactly once per partition
                # half; the other heads' first writes then init via the
                # pending-zero overwrite instead of accumulating garbage.
                for h in range(H):
                    nc.tensor.matmul(
                        ctx_ps[
                            (h // 8) * 64:(h // 8) * 64 + 64,
                            (h % 8) * 64:(h % 8) * 64 + 64,
                        ],
                        vg_t[:, h * D:(h + 1) * D],
                        kf_t[:, h * D:(h + 1) * D],
                        start=(gc == 0 and h % 8 == 0),
                        stop=(gc == NCH - 1),
                        skip_group_check=True,
                    )

            for blk in range(4):
                # input DMAs split by n-half so the first matmul of the
                # block is gated on half the bytes; at blk 0 interleave the
                # w1/b1 emissions so all gating loads land on the earliest-
                # starting DMA queues.
                if fp8:
                    xt8_in = xt8_pool.tile([128, 8, C], FP8, name="xt8_in", tag="xt8")
                    src8 = xt8.rearrange("(k p) n -> p k n", p=128)
                    # w1 arrives in m-order so gate1 m=0 is gated on 128KB,
                    # not the whole 1MB; interleaved with the xt8 parts
                    w1m = w1.rearrange("p (k m) -> p k m", k=8)
                    for hf in range(2):
                        for kq in range(4):
                            if blk == 0:
                                i = hf * 4 + kq
                                nc.sync.dma_start(
                                    w1_sb[:, :, i * 128:(i + 1) * 128],
                                    w1m[:, :, i * 128:(i + 1) * 128],
                                )
                            nc.sync.dma_start(
                                xt8_in[:, kq * 2:(kq + 1) * 2,
                                       hf * 512:(hf + 1) * 512],
                                src8[:, kq * 2:(kq + 1) * 2,
                                     blk * 1024 + hf * 512:
                                     blk * 1024 + (hf + 1) * 512],
                            )
                    if blk == 0:
                        nc.sync.dma_start(b1_sb, b1s)
                elif blk == 0:
                    dma_split(w1_sb, w1.rearrange("p (k m) -> p k m", k=8), 8)
                    nc.sync.dma_start(b1_sb, b1s)
                xt_in = xt_pool.tile([128, 8, C], FP16, name="xt_in", tag="xt")
                srcx = xt.rearrange("(k p) n -> p k n", p=128)
                for hf in range(2):
                    for kq in range(2):
                        nc.sync.dma_start(
                            xt_in[:, kq * 4:(kq + 1) * 4,
                                  hf * 512:(hf + 1) * 512],
                            srcx[:, kq * 4:(kq + 1) * 4,
                                 blk * 1024 + hf * 512:
                                 blk * 1024 + (hf + 1) * 512],
                        )

                # ---- gate1: hT[m-tile, n] = relu(x@w1+b1).T ----
                ht = ht_pool.tile([128, 8, C], HDT, name="ht", tag="ht")
                for m in range(8):
                    pss = [
                        gps_pool.tile([128, 512], F32, name="g1ps", tag="gps")
                        for _ in range(2)
                    ]
                    if fp8:
                        for kp in range(4):
                            lhs = w1_sb[:, 2 * kp:2 * kp + 2, m * 128:(m + 1) * 128]
                            for half in range(2):
                                nc.tensor.matmul(
                                    pss[half],
                                    lhs,
                                    xt8_in[:, 2 * kp:2 * kp + 2,
                                           half * 512:(half + 1) * 512],
                                    start=(kp == 0),
                                    stop=(kp == 3),
                                    perf_mode=DR,
                                )
                    else:
                        for k in range(8):
                            lhs = w1_sb[:, k, m * 128:(m + 1) * 128]
                            for half in range(2):
                                nc.tensor.matmul(
                                    pss[half],
                                    lhs,
                                    xt_in[:, k, half * 512:(half + 1) * 512],
                                    start=(k == 0),
                                    stop=(k == 7),
                                )
                    for half in range(2):
                        nc.scalar.activation(
                            ht[:, m, half * 512:(half + 1) * 512],
                            pss[half],
                            AF.Relu,
                            bias=b1_sb[:, m:m + 1],
                            scale=g1_scale,
                        )

                if blk == 0:
                    # w2/wkv arrive during block 0's gate1; xq prefetches after
                    emit_deferred_consts()
                # prefetch phase-B xq tiles while DMA is quiet
                if blk >= 1:
                    bx_tiles[blk - 1] = emit_bxq_dma(blk - 1)
                    if blk == 3:
                        bx_tiles[3] = emit_bxq_dma(3)

                # ---- per chunk: gate2 -> kv -> (delayed) ctx ----
                for c in range(8):
                    gc = blk * 8 + c
                    gt = g_pool.tile([128, C], FP16, name="gt", tag="gt")
                    for t in range(2):
                        ps2 = gps_pool.tile([128, 512], F32, name="g2ps", tag="gps")
                        if fp8:
                            for kp in range(4):
                                nc.tensor.matmul(
                                    ps2,
                                    ht[:, 2 * kp:2 * kp + 2, c * 128:(c + 1) * 128],
                                    w2_sb[:, 2 * kp:2 * kp + 2,
                                          t * 512:(t + 1) * 512],
                                    start=(kp == 0),
                                    stop=(kp == 3 and not with_bias),
                                    perf_mode=DR,
                                )
                        else:
                            for k in range(8):
                                nc.tensor.matmul(
                                    ps2,
                                    ht[:, k, c * 128:(c + 1) * 128],
                                    w2_sb[:, k, t * 512:(t + 1) * 512],
                                    start=(k == 0),
                                    stop=(k == 7 and not with_bias),
                                )
                        if with_bias:
                            nc.tensor.matmul(
                                ps2,
                                ones_r,
                                b2_r[:, t * 512:(t + 1) * 512],
                                start=False,
                                stop=True,
                            )
                        nc.scalar.activation(
                            gt[:, t * 512:(t + 1) * 512], ps2, AF.Sigmoid,
                            scale=g2_scale,
                        )

                    # kv projection for this chunk; k and v psum halves
                    ps_k = kvps_pool.tile([128, C], F32, name="ps_k", tag="kvps")
                    ps_v = kvps_pool.tile([128, C], F32, name="ps_v", tag="kvps")
                    for k in range(8):
                        lhs = xt_in[:, k, c * 128:(c + 1) * 128]
                        for t in range(2):
                            nc.tensor.matmul(
                                ps_k[:, t * 512:(t + 1) * 512],
                                lhs,
                                wkv_sb[:, k, t * 512:(t + 1) * 512],
                                start=(k == 0),
                                stop=(k == 7),
                            )
                        for t in range(2):
                            nc.tensor.matmul(
                                ps_v[:, t * 512:(t + 1) * 512],
                                lhs,
                                wkv_sb[:, k, C + t * 512:C + (t + 1) * 512],
                                start=(k == 0),
                                stop=(k == 7),
                            )
                    kf = kf_pool.tile([128, C], FP16, name="kf", tag="kf")
                    nc.scalar.copy(kf, ps_k)
                    vg = vg_pool.tile([128, C], FP16, name="vg", tag="vg")
                    nc.vector.tensor_mul(vg, ps_v, gt)

                    # ctx for the PREVIOUS chunk (kf/vg conversions for it
                    # ran while this chunk's kv matmuls streamed)
                    if pending:
                        emit_ctx(*pending.pop(0))
                    pending.append((kf, vg, gc))

            while pending:
                emit_ctx(*pending.pop(0))

        # =========================================================
        # Softmax over d (free dim of ctxT) + block-diag S pairs
        # =========================================================
        with ExitStack() as sm:
            smp = sm.enter_context(tc.tile_pool(name="smpool", bufs=1))
            smps = sm.enter_context(tc.tile_pool(name="smps", bufs=4, space="PSUM"))
            maxs = smp.tile([128, 8], F32, name="maxs")
            nc.vector.tensor_reduce(
                maxs,
                ctx_ps.rearrange("p (b d) -> p b d", b=8),
                axis=mybir.AxisListType.X,
                op=mybir.AluOpType.max,
            )
            cmx = smp.tile([128, 512], F32, name="cmx")
            nc.vector.tensor_sub(
                cmx.rearrange("p (h d) -> p h d", h=8),
                ctx_ps.rearrange("p (h d) -> p h d", h=8),
                maxs.unsqueeze(-1).broadcast_to([128, 8, 64]),
            )
            et = smp.tile([128, 512], F32, name="et")
            nc.scalar.activation(et, cmx, AF.Exp, scale=float(SCALE))
            sums = smp.tile([128, 8], F32, name="sums")
            nc.vector.tensor_reduce(
                sums,
                et.rearrange("p (b d) -> p b d", b=8),
                axis=mybir.AxisListType.X,
                op=mybir.AluOpType.add,
            )
            recs = smp.tile([128, 8], F32, name="recs")
            nc.vector.reciprocal(recs, sums)
            st = smp.tile([128, 512], F32, name="st")
            nc.vector.tensor_mul(
                st.rearrange("p (h d) -> p h d", h=8),
                et.rearrange("p (h d) -> p h d", h=8),
                recs.unsqueeze(-1).broadcast_to([128, 8, 64]),
            )
            # st rows e (64 per half), cols d per head.  Transposing the
            # side-by-side pair [ctxT_2j | ctxT_2j+1] ([64, 128]) gives
            # [S_2j stacked above S_2j+1] ([128, 64]); scatter block-diag.
            for j in range(8):
                half = j // 4  # heads 0-7 in lower partitions, 8-15 upper
                base = half * 64
                colj = (2 * j) % 8
                tp = smps.tile([128, 64], F32, name="smtp", tag="smtp")
                nc.tensor.transpose(
                    tp,
                    st[base:base + 64, colj * 64:(colj + 2) * 64],
                    ident_sb[base:base + 64, :],
                )
                if j % 2 == 0:
                    nc.vector.tensor_copy(spairs[j][0:64, 0:64], tp[0:64, :])
                    nc.vector.tensor_copy(spairs[j][64:128, 64:128], tp[64:128, :])
                else:
                    nc.scalar.copy(spairs[j][0:64, 0:64], tp[0:64, :])
                    nc.scalar.copy(spairs[j][64:128, 64:128], tp[64:128, :])

        # =========================================================
        # Phase B: o[nchunk, j*128:(j+1)*128] = q_pair @ blockdiag(S)
        # =========================================================
        with ExitStack() as pb:
            oo_pool = pb.enter_context(tc.tile_pool(name="bo", bufs=8))
            bops_pool = pb.enter_context(
                tc.tile_pool(name="bops", bufs=7, space="PSUM")
            )
            for blk in range(4):
                bx = bx_tiles.pop(blk)
                for c4 in range(8):
                    oo = oo_pool.tile([128, C], FP16, name="oo", tag="oo")
                    nch = blk * 8 + c4
                    for half in range(2):
                        ops = bops_pool.tile([128, 512], F32, name="ops", tag="ops")
                        for jj in range(4):
                            j = half * 4 + jj
                            nc.tensor.matmul(
                                ops[:, jj * 128:(jj + 1) * 128],
                                bx[:, j, c4 * 128:(c4 + 1) * 128],
                                spairs[j],
                                start=True,
                                stop=True,
                                skip_group_check=True,
                            )
                        if half == 0:
                            nc.vector.tensor_copy(
                                oo[:, half * 512:(half + 1) * 512], ops
                            )
                        else:
                            nc.scalar.copy(
                                oo[:, half * 512:(half + 1) * 512], ops
                            )
                    nc.sync.dma_start(o[nch * 128:(nch + 1) * 128, :], oo)

    nc.compile()
    return nc


def _get_program(gate_mode=None, with_bias=False):
    if gate_mode is None:
        gate_mode = GATE_MODE
    key = (gate_mode, bool(with_bias))
    if key not in _CACHE:
        _CACHE[key] = _build_program(gate_mode, with_bias)
    return _CACHE[key]


def make_in_maps(x1, x2, Wkv1, Wkv2, g1_w1, g1_b1, g1_w2, g1_b2,
                 g2_w1, g2_b1, g2_w2, g2_b2, gate_mode=None):
    """Core (s, b): cores 0-3 = (s=0, b), cores 4-7 = (s=1, b)."""
    import ml_dtypes
    if gate_mode is None:
        gate_mode = GATE_MODE
    fp8 = gate_mode == "fp8"
    F8 = ml_dtypes.float8_e4m3
    ident = np.vstack([np.eye(64, dtype=np.float32)] * 2)

    def dev_w(w):
        # [k*128+p, m] -> [p, k*M+m] (SBUF layout, contiguous DMA lines)
        M = w.shape[1]
        return np.ascontiguousarray(
            w.reshape(8, 128, M).transpose(1, 0, 2).reshape(128, 8 * M)
        )

    def prep_stream(x, wkv, w1, b1, w2, b2):
        m = {
            "xt": x.T.astype(np.float16, order="C"),
            "wkv": dev_w(wkv.astype(np.float16)),
            "ident": ident,
        }
        if fp8:
            m["xt8"] = (x.T * S_X).astype(F8, order="C")
            m["w1"] = dev_w((w1 * S_W).astype(F8))
            m["w2"] = dev_w((w2 * S_W).astype(F8))
            m["b1s"] = np.ascontiguousarray((S_H * b1).reshape(8, 128).T)
        else:
            m["w1"] = dev_w(w1.astype(np.float16))
            m["w2"] = dev_w(w2.astype(np.float16))
            m["b1s"] = np.ascontiguousarray(b1.reshape(8, 128).T)
        m["b2r"] = b2.reshape(1, C).astype(np.float16)
        return m

    in_maps = []
    for core in range(8):
        s, b = core // 4, core % 4
        if s == 0:
            m = prep_stream(x1[b], Wkv1, g1_w1, g1_b1, g1_w2, g1_b2)
            m["xqt"] = x2[b].T.astype(np.float16, order="C")
        else:
            m = prep_stream(x2[b], Wkv2, g2_w1, g2_b1, g2_w2, g2_b2)
            m["xqt"] = x1[b].T.astype(np.float16, order="C")
        in_maps.append(m)
    return in_maps


def kernel(x1, x2, Wkv1, Wkv2, g1_w1, g1_b1, g1_w2, g1_b2,
           g2_w1, g2_b1, g2_w2, g2_b2, _runner=None):
    """Full-input entry point.  Returns (o1, o2), each [4, 4096, 1024] f32."""
    from concourse.bass_utils import run_bass_kernel_spmd

    args = [np.asarray(a, dtype=np.float32) for a in
            (x1, x2, Wkv1, Wkv2, g1_w1, g1_b1, g1_w2, g1_b2,
             g2_w1, g2_b1, g2_w2, g2_b2)]
    with_bias = bool(np.any(args[7]) or np.any(args[11]))  # g1_b2, g2_b2
    nc = _get_program(GATE_MODE, with_bias)
    in_maps = make_in_maps(*args)
    if not with_bias:
        for m in in_maps:
            m.pop("b2r", None)
    if _runner is None:
        res = run_bass_kernel_spmd(nc, in_maps, core_ids=list(range(8)))
        results = res.results
    else:
        results = _runner(nc, in_maps)

    B = x1.shape[0]
    o1 = np.empty((B, N, C), dtype=np.float32)
    o2 = np.empty((B, N, C), dtype=np.float32)
    for core in range(8):
        s, b = core // 4, core % 4
        out = np.asarray(results[core]["o"], dtype=np.float32)
        if s == 0:
            o2[b] = out   # core projected x1 -> ctx1 -> o2 = q2 @ ctx1
        else:
            o1[b] = out
    return (o1, o2)


# revision 58
# speedup vs baseline: 1.0011x; 1.0011x over previous
"""Trainium2 Bass kernel for nn_CrossAttention (dense_transformer).

Reference computation (per batch b, per stream s in {1,2}):
    q_s   = heads(x_s)                      # [H, N, D] slices of x_s
    kv_s  = x_s @ Wkv_s -> k_s, v_s         # [N, C] each
    gate_s= sigmoid(relu(x_s @ w1 + b1) @ w2 + b2)
    ctx_s = softmax_d( scale * k_s^T @ (v_s * gate_s) )   # [H, D, D]
    o_1   = q_1 @ ctx_2 ; o_2 = q_2 @ ctx_1  (cross)

Sharding: 8 cores = (stream s, batch b) pairs.  Core (s, b) projects
x_s[b] (kv + gate + ctx_s[b]) and then computes the OTHER stream's
output o_{1-s}[b] = q_{1-s}[b] @ softmax(ctx_s[b]).  No cross-core
communication; host concatenates outputs.

v2: host pre-transposes/pre-casts x (fp16), so the device does no
transposes and no DRAM spills.  All GEMMs fp16 (1 cycle/row); the gate
MLP can optionally run fp8e4 DoubleRow (0.5 cycles/row).  ctx is
accumulated in PSUM across all 32 n-chunks (two 8-head groups stacked
on partition halves -> one PSUM bank).
"""

import numpy as np
from contextlib import ExitStack

N = 4096
C = 1024
H = 16
D = 64
SCALE = D ** (-0.5)
NCH = N // 128       # 32 n-chunks of 128 rows

GATE_MODE = "fp8"    # 'fp16' | 'fp8'
S_X = 16.0           # fp8 activation scale for x
S_W = 256.0          # fp8 weight scale
S_H = 32.0           # fp8 scale for hidden h

_CACHE = {}


def _build_program(gate_mode, with_bias):
    import concourse.bass as bass
    import concourse.bacc as bacc
    import concourse.tile as tile
    import concourse.mybir as mybir

    F32 = mybir.dt.float32
    FP16 = mybir.dt.float16
    FP8 = mybir.dt.float8e4
    AF = mybir.ActivationFunctionType
    DR = mybir.MatmulPerfMode.DoubleRow
    fp8 = gate_mode == "fp8"
    HDT = FP8 if fp8 else FP16

    nc = bacc.Bacc("TRN2", target_bir_lowering=False, debug=False, num_devices=8)

    # weights arrive host-rearranged to the SBUF layout [p, k, m] so the
    # DMA is a straight contiguous copy (8-16KB lines per partition)
    xt = nc.dram_tensor("xt", [C, N], FP16, kind="ExternalInput").ap()
    xqt = nc.dram_tensor("xqt", [C, N], FP16, kind="ExternalInput").ap()
    wkv = nc.dram_tensor("wkv", [128, 8 * 2 * C], FP16, kind="ExternalInput").ap()
    w1 = nc.dram_tensor("w1", [128, 8 * C], HDT, kind="ExternalInput").ap()
    w2 = nc.dram_tensor("w2", [128, 8 * C], HDT, kind="ExternalInput").ap()
    b1s = nc.dram_tensor("b1s", [128, 8], F32, kind="ExternalInput").ap()
    ident = nc.dram_tensor("ident", [128, 64], F32, kind="ExternalInput").ap()
    if fp8:
        xt8 = nc.dram_tensor("xt8", [C, N], FP8, kind="ExternalInput").ap()
    if with_bias:
        b2r = nc.dram_tensor("b2r", [1, C], FP16, kind="ExternalInput").ap()
    o = nc.dram_tensor("o", [N, C], FP16, kind="ExternalOutput").ap()

    # activation post-scales to undo the fp8 pre-scales
    g1_scale = (S_H / (S_X * S_W)) if fp8 else 1.0
    g2_scale = (1.0 / (S_H * S_W)) if fp8 else 1.0
    ones_val = (S_H * S_W) if fp8 else 1.0

    with tile.TileContext(nc) as tc, ExitStack() as ctx:
        # ---------- persistent constants ----------
        # DMA order matters: only w1/b1 (+ first x block) gate the first
        # matmul, so emit those first and defer w2/wkv into block 0's
        # compute window.
        # big DMAs are split into k-range parts so they spread across DMA
        # queues (a single dma_start lands on one queue)
        def dma_split(dst, src, parts):
            kk = 8 // parts
            for i in range(parts):
                nc.sync.dma_start(
                    dst[:, i * kk:(i + 1) * kk, :], src[:, i * kk:(i + 1) * kk, :]
                )

        cpool = ctx.enter_context(tc.tile_pool(name="consts", bufs=1))
        w1_sb = cpool.tile([128, 8, C], HDT, name="w1_sb")
        b1_sb = cpool.tile([128, 8], F32, name="b1_sb")
        w2_sb = cpool.tile([128, 8, C], HDT, name="w2_sb")
        wkv_sb = cpool.tile([128, 8, 2 * C], FP16, name="wkv_sb")
        ident_sb = cpool.tile([128, 64], F32, name="ident_sb")

        def emit_deferred_consts():
            dma_split(w2_sb, w2.rearrange("p (k m) -> p k m", k=8), 4)
            dma_split(wkv_sb, wkv.rearrange("p (k m) -> p k m", k=8), 8)
            nc.sync.dma_start(ident_sb, ident)
        if with_bias:
            ones_sb = cpool.tile([1, 128], F32, name="ones_sb")
            nc.vector.memset(ones_sb, ones_val)
            ones_r = cpool.tile([1, 128], FP16, name="ones_r")
            nc.vector.tensor_copy(ones_r, ones_sb)
            b2_r = cpool.tile([1, C], FP16, name="b2_r")
            nc.sync.dma_start(b2_r, b2r)

        # pre-warm the scalar Exp activation table while the engine is idle
        # so the softmax doesn't pay the ~1.3us ACT_TABLE_LOAD at its start
        warm = cpool.tile([1, 16], F32, name="warm")
        nc.vector.memset(warm, 0.0)
        warm2 = cpool.tile([1, 16], F32, name="warm2")
        nc.scalar.activation(warm2, warm, AF.Exp)

        # spair off-diagonal blocks are zero forever: zero them once here,
        # off the critical path, so softmax only writes the diagonals
        spool = ctx.enter_context(tc.tile_pool(name="spairs", bufs=1))
        spairs = [spool.tile([128, 128], FP16, name=f"spair{j}") for j in range(8)]
        for j in range(8):
            nc.vector.memset(spairs[j], 0.0)

        # ctx accumulator in PSUM: heads 0-7 on partitions 0-63, heads
        # 8-15 on 64-127; head h at cols (h%8)*64, layout [e, d].
        ctxps_pool = ctx.enter_context(
            tc.tile_pool(name="ctxps", bufs=1, space="PSUM")
        )
        ctx_ps = ctxps_pool.tile([128, 512], F32, name="ctx_ps")

        # phase-B xq tiles live alongside phase A so DMA prefetch overlaps
        bxq_pool = ctx.enter_context(tc.tile_pool(name="bxq", bufs=4))

        def emit_bxq_dma(blk):
            bx = bxq_pool.tile([128, 8, 1024], FP16, name="bx", tag="bx")
            src = xqt.rearrange("(j p) n -> p j n", p=128)[
                :, :, blk * 1024:(blk + 1) * 1024
            ]
            for i in range(4):
                nc.sync.dma_start(bx[:, 2 * i:2 * i + 2, :], src[:, 2 * i:2 * i + 2, :])
            return bx

        # =========================================================
        # Phase A: gates + kv projection + ctx accumulation, fused
        # =========================================================
        with ExitStack() as pa:
            xt_pool = pa.enter_context(tc.tile_pool(name="xt", bufs=2))
            if fp8:
                xt8_pool = pa.enter_context(tc.tile_pool(name="xt8", bufs=2))
            ht_pool = pa.enter_context(tc.tile_pool(name="ht", bufs=2))
            g_pool = pa.enter_context(tc.tile_pool(name="g", bufs=3))
            kf_pool = pa.enter_context(tc.tile_pool(name="kf", bufs=4))
            vg_pool = pa.enter_context(tc.tile_pool(name="vg", bufs=4))
            gps_pool = pa.enter_context(
                tc.tile_pool(name="gps", bufs=3, space="PSUM")
            )
            kvps_pool = pa.enter_context(
                tc.tile_pool(name="kvps", bufs=2, space="PSUM")
            )

            bx_tiles = {}
            pending = []  # (kf, vg, global_chunk) awaiting ctx matmuls

            def emit_ctx(kf_t, vg_t, gc):
                # start=True marks the whole 2KB PSUM bank (per partition)
                # as pending-zero, so issue it exactly once per partition
                # half; the other heads' first writes then init via the
                # pending-zero overwrite instead of accumulating garbage.
                for h in range(H):
                    nc.tensor.matmul(
                        ctx_ps[
                            (h // 8) * 64:(h // 8) * 64 + 64,
                            (h % 8) * 64:(h % 8) * 64 + 64,
                        ],
                        vg_t[:, h * D:(h + 1) * D],
                        kf_t[:, h * D:(h + 1) * D],
                        start=(gc == 0 and h % 8 == 0),
                        stop=(gc == NCH - 1),
                        skip_group_check=True,
                    )

            for blk in range(4):
                # input DMAs split by n-half so the first matmul of the
                # block is gated on half the bytes; at blk 0 interleave the
                # w1/b1 emissions so all gating loads land on the earliest-
                # starting DMA queues.
                if fp8:
                    xt8_in = xt8_pool.tile([128, 8, C], FP8, name="xt8_in", tag="xt8")
                    src8 = xt8.rearrange("(k p) n -> p k n", p=128)
                    # w1 arrives in m-order so gate1 m=0 is gated on 128KB,
                    # not the whole 1MB; interleaved with the xt8 parts
                    # kq-outer order matches gate1's kp-ascending consumption,
                    # so the first accumulation chain starts after ~0.3MB
                    w1m = w1.rearrange("p (k m) -> p k m", k=8)
                    for kq in range(4):
                        for hf in range(2):
                            if blk == 0:
                                i = kq * 2 + hf
                                nc.sync.dma_start(
                                    w1_sb[:, :, i * 128:(i + 1) * 128],
                                    w1m[:, :, i * 128:(i + 1) * 128],
                                )
                            nc.sync.dma_start(
                                xt8_in[:, kq * 2:(kq + 1) * 2,
                                       hf * 512:(hf + 1) * 512],
                                src8[:, kq * 2:(kq + 1) * 2,
                                     blk * 1024 + hf * 512:
                                     blk * 1024 + (hf + 1) * 512],
                            )
                    if blk == 0:
                        nc.sync.dma_start(b1_sb, b1s)
                elif blk == 0:
                    dma_split(w1_sb, w1.rearrange("p (k m) -> p k m", k=8), 8)
                    nc.sync.dma_start(b1_sb, b1s)
                xt_in = xt_pool.tile([128, 8, C], FP16, name="xt_in", tag="xt")
                srcx = xt.rearrange("(k p) n -> p k n", p=128)
                for hf in range(2):
                    for kq in range(2):
                        nc.sync.dma_start(
                            xt_in[:, kq * 4:(kq + 1) * 4,
                                  hf * 512:(hf + 1) * 512],
                            srcx[:, kq * 4:(kq + 1) * 4,
                                 blk * 1024 + hf * 512:
                                 blk * 1024 + (hf + 1) * 512],
                        )

                # ---- gate1: hT[m-tile, n] = relu(x@w1+b1).T ----
                ht = ht_pool.tile([128, 8, C], HDT, name="ht", tag="ht")
                for m in range(8):
                    pss = [
                        gps_pool.tile([128, 512], F32, name="g1ps", tag="gps")
                        for _ in range(2)
                    ]
                    if fp8:
                        for kp in range(4):
                            lhs = w1_sb[:, 2 * kp:2 * kp + 2, m * 128:(m + 1) * 128]
                            for half in range(2):
                                nc.tensor.matmul(
                                    pss[half],
                                    lhs,
                                    xt8_in[:, 2 * kp:2 * kp + 2,
                                           half * 512:(half + 1) * 512],
                                    start=(kp == 0),
                                    stop=(kp == 3),
                                    perf_mode=DR,
                                )
                    else:
                        for k in range(8):
                            lhs = w1_sb[:, k, m * 128:(m + 1) * 128]
                            for half in range(2):
                                nc.tensor.matmul(
                                    pss[half],
                                    lhs,
                                    xt_in[:, k, half * 512:(half + 1) * 512],
                                    start=(k == 0),
                                    stop=(k == 7),
                                )
                    for half in range(2):
                        nc.scalar.activation(
                            ht[:, m, half * 512:(half + 1) * 512],
                            pss[half],
                            AF.Relu,
                            bias=b1_sb[:, m:m + 1],
                            scale=g1_scale,
                        )

                if blk == 0:
                    # w2/wkv arrive during block 0's gate1; xq prefetches after
                    emit_deferred_consts()
                # prefetch phase-B xq tiles while DMA is quiet
                if blk >= 1:
                    bx_tiles[blk - 1] = emit_bxq_dma(blk - 1)
                    if blk == 3:
                        bx_tiles[3] = emit_bxq_dma(3)

                # ---- per chunk: gate2 -> kv -> (delayed) ctx ----
                for c in range(8):
                    gc = blk * 8 + c
                    gt = g_pool.tile([128, C], FP16, name="gt", tag="gt")
                    for t in range(2):
                        ps2 = gps_pool.tile([128, 512], F32, name="g2ps", tag="gps")
                        if fp8:
                            for kp in range(4):
                                nc.tensor.matmul(
                                    ps2,
                                    ht[:, 2 * kp:2 * kp + 2, c * 128:(c + 1) * 128],
                                    w2_sb[:, 2 * kp:2 * kp + 2,
                                          t * 512:(t + 1) * 512],
                                    start=(kp == 0),
                                    stop=(kp == 3 and not with_bias),
                                    perf_mode=DR,
                                )
                        else:
                            for k in range(8):
                                nc.tensor.matmul(
                                    ps2,
                                    ht[:, k, c * 128:(c + 1) * 128],
                                    w2_sb[:, k, t * 512:(t + 1) * 512],
                                    start=(k == 0),
                                    stop=(k == 7 and not with_bias),
                                )
                        if with_bias:
                            nc.tensor.matmul(
                                ps2,
                                ones_r,
                                b2_r[:, t * 512:(t + 1) * 512],
                                start=False,
                                stop=True,
                            )
                        nc.scalar.activation(
                            gt[:, t * 512:(t + 1) * 512], ps2, AF.Sigmoid,
                            scale=g2_scale,
                        )

                    # kv projection for this chunk; k and v psum halves
                    ps_k = kvps_pool.tile([128, C], F32, name="ps_k", tag="kvps")
                    ps_v = kvps_pool.tile([128, C], F32, name="ps_v", tag="kvps")
                    for k in range(8):
                        lhs = xt_in[:, k, c * 128:(c + 1) * 128]
                        for t in range(2):
                            nc.tensor.matmul(
                                ps_k[:, t * 512:(t + 1) * 512],
                                lhs,
                                wkv_sb[:, k, t * 512:(t + 1) * 512],
                                start=(k == 0),
                                stop=(k == 7),
                            )
                        for t in range(2):
                            nc.tensor.matmul(
                                ps_v[:, t * 512:(t + 1) * 512],
                                lhs,
                                wkv_sb[:, k, C + t * 512:C + (t + 1) * 512],
                                start=(k == 0),
                                stop=(k == 7),
                            )
                    kf = kf_pool.tile([128, C], FP16, name="kf", tag="kf")
                    nc.scalar.copy(kf, ps_k)
                    vg = vg_pool.tile([128, C], FP16, name="vg", tag="vg")
                    nc.vector.tensor_mul(vg, ps_v, gt)

                    # ctx lags two chunks and is emitted in pairs: the kf/vg
                    # conversions overlap the kv matmuls, and batching two
                    # chunks of tiny ctx matmuls halves the PE tile-mode
                    # transitions (~200ns each)
                    pending.append((kf, vg, gc))
                    if len(pending) >= 4:
                        emit_ctx(*pending.pop(0))
                        emit_ctx(*pending.pop(0))

            while pending:
                emit_ctx(*pending.pop(0))

        # =========================================================
        # Softmax over d (free dim of ctxT) + block-diag S pairs
        # =========================================================
        with ExitStack() as sm:
            smp = sm.enter_context(tc.tile_pool(name="smpool", bufs=1))
            smps = sm.enter_context(tc.tile_pool(name="smps", bufs=4, space="PSUM"))
            maxs = smp.tile([128, 8], F32, name="maxs")
            nc.vector.tensor_reduce(
                maxs,
                ctx_ps.rearrange("p (b d) -> p b d", b=8),
                axis=mybir.AxisListType.X,
                op=mybir.AluOpType.max,
            )
            cmx = smp.tile([128, 512], F32, name="cmx")
            nc.vector.tensor_sub(
                cmx.rearrange("p (h d) -> p h d", h=8),
                ctx_ps.rearrange("p (h d) -> p h d", h=8),
                maxs.unsqueeze(-1).broadcast_to([128, 8, 64]),
            )
            et = smp.tile([128, 512], F32, name="et")
            nc.scalar.activation(et, cmx, AF.Exp, scale=float(SCALE))
            sums = smp.tile([128, 8], F32, name="sums")
            nc.vector.tensor_reduce(
                sums,
                et.rearrange("p (b d) -> p b d", b=8),
                axis=mybir.AxisListType.X,
                op=mybir.AluOpType.add,
            )
            recs = smp.tile([128, 8], F32, name="recs")
            nc.vector.reciprocal(recs, sums)
            st = smp.tile([128, 512], F32, name="st")
            nc.vector.tensor_mul(
                st.rearrange("p (h d) -> p h d", h=8),
                et.rearrange("p (h d) -> p h d", h=8),
                recs.unsqueeze(-1).broadcast_to([128, 8, 64]),
            )
            # st rows e (64 per half), cols d per head.  Transposing the
            # side-by-side pair [ctxT_2j | ctxT_2j+1] ([64, 128]) gives
            # [S_2j stacked above S_2j+1] ([128, 64]); scatter block-diag.
            for j in range(8):
                half = j // 4  # heads 0-7 in lower partitions, 8-15 upper
                base = half * 64
                colj = (2 * j) % 8
                tp = smps.tile([128, 64], F32, name="smtp", tag="smtp")
                nc.tensor.transpose(
                    tp,
                    st[base:base + 64, colj * 64:(colj + 2) * 64],
                    ident_sb[base:base + 64, :],
                )
                if j % 2 == 0:
                    nc.vector.tensor_copy(spairs[j][0:64, 0:64], tp[0:64, :])
                    nc.vector.tensor_copy(spairs[j][64:128, 64:128], tp[64:128, :])
                else:
                    nc.scalar.copy(spairs[j][0:64, 0:64], tp[0:64, :])
                    nc.scalar.copy(spairs[j][64:128, 64:128], tp[64:128, :])

        # =========================================================
        # Phase B: o[nchunk, j*128:(j+1)*128] = q_pair @ blockdiag(S)
        # =========================================================
        with ExitStack() as pb:
            oo_pool = pb.enter_context(tc.tile_pool(name="bo", bufs=8))
            bops_pool = pb.enter_context(
                tc.tile_pool(name="bops", bufs=7, space="PSUM")
            )
            for blk in range(4):
                bx = bx_tiles.pop(blk)
                for c4 in range(8):
                    oo = oo_pool.tile([128, C], FP16, name="oo", tag="oo")
                    nch = blk * 8 + c4
                    for half in range(2):
                        ops = bops_pool.tile([128, 512], F32, name="ops", tag="ops")
                        for jj in range(4):
                            j = half * 4 + jj
                            nc.tensor.matmul(
                                ops[:, jj * 128:(jj + 1) * 128],
                                bx[:, j, c4 * 128:(c4 + 1) * 128],
                                spairs[j],
                                start=True,
                                stop=True,
                                skip_group_check=True,
                            )
                        if half == 0:
                            nc.vector.tensor_copy(
                                oo[:, half * 512:(half + 1) * 512], ops
                            )
                        else:
                            nc.scalar.copy(
                                oo[:, half * 512:(half + 1) * 512], ops
                            )
                    nc.sync.dma_start(o[nch * 128:(nch + 1) * 128, :], oo)

    nc.compile()
    return nc


def _get_program(gate_mode=None, with_bias=False):
    if gate_mode is None:
        gate_mode = GATE_MODE
    key = (gate_mode, bool(with_bias))
    if key not in _CACHE:
        _CACHE[key] = _build_program(gate_mode, with_bias)
    return _CACHE[key]


def make_in_maps(x1, x2, Wkv1, Wkv2, g1_w1, g1_b1, g1_w2, g1_b2,
                 g2_w1, g2_b1, g2_w2, g2_b2, gate_mode=None):
    """Core (s, b): cores 0-3 = (s=0, b), cores 4-7 = (s=1, b)."""
    import ml_dtypes
    if gate_mode is None:
        gate_mode = GATE_MODE
    fp8 = gate_mode == "fp8"
    F8 = ml_dtypes.float8_e4m3
    ident = np.vstack([np.eye(64, dtype=np.float32)] * 2)

    def dev_w(w):
        # [k*128+p, m] -> [p, k*M+m] (SBUF layout, contiguous DMA lines)
        M = w.shape[1]
        return np.ascontiguousarray(
            w.reshape(8, 128, M).transpose(1, 0, 2).reshape(128, 8 * M)
        )

    def prep_stream(x, wkv, w1, b1, w2, b2):
        m = {
            "xt": x.T.astype(np.float16, order="C"),
            "wkv": dev_w(wkv.astype(np.float16)),
            "ident": ident,
        }
        if fp8:
            m["xt8"] = (x.T * S_X).astype(F8, order="C")
            m["w1"] = dev_w((w1 * S_W).astype(F8))
            m["w2"] = dev_w((w2 * S_W).astype(F8))
            m["b1s"] = np.ascontiguousarray((S_H * b1).reshape(8, 128).T)
        else:
            m["w1"] = dev_w(w1.astype(np.float16))
            m["w2"] = dev_w(w2.astype(np.float16))
            m["b1s"] = np.ascontiguousarray(b1.reshape(8, 128).T)
        m["b2r"] = b2.reshape(1, C).astype(np.float16)
        return m

    in_maps = []
    for core in range(8):
        s, b = core // 4, core % 4
        if s == 0:
            m = prep_stream(x1[b], Wkv1, g1_w1, g1_b1, g1_w2, g1_b2)
            m["xqt"] = x2[b].T.astype(np.float16, order="C")
        else:
            m = prep_stream(x2[b], Wkv2, g2_w1, g2_b1, g2_w2, g2_b2)
            m["xqt"] = x1[b].T.astype(np.float16, order="C")
        in_maps.append(m)
    return in_maps


def kernel(x1, x2, Wkv1, Wkv2, g1_w1, g1_b1, g1_w2, g1_b2,
           g2_w1, g2_b1, g2_w2, g2_b2, _runner=None):
    """Full-input entry point.  Returns (o1, o2), each [4, 4096, 1024] f32."""
    from concourse.bass_utils import run_bass_kernel_spmd

    args = [np.asarray(a, dtype=np.float32) for a in
            (x1, x2, Wkv1, Wkv2, g1_w1, g1_b1, g1_w2, g1_b2,
             g2_w1, g2_b1, g2_w2, g2_b2)]
    with_bias = bool(np.any(args[7]) or np.any(args[11]))  # g1_b2, g2_b2
    nc = _get_program(GATE_MODE, with_bias)
    in_maps = make_in_maps(*args)
    if not with_bias:
        for m in in_maps:
            m.pop("b2r", None)
    if _runner is None:
        res = run_bass_kernel_spmd(nc, in_maps, core_ids=list(range(8)))
        results = res.results
    else:
        results = _runner(nc, in_maps)

    B = x1.shape[0]
    o1 = np.empty((B, N, C), dtype=np.float32)
    o2 = np.empty((B, N, C), dtype=np.float32)
    for core in range(8):
        s, b = core // 4, core % 4
        out = np.asarray(results[core]["o"], dtype=np.float32)
        if s == 0:
            o2[b] = out   # core projected x1 -> ctx1 -> o2 = q2 @ ctx1
        else:
            o1[b] = out
    return (o1, o2)
